# revision 11
# baseline (speedup 1.0000x reference)
"""ClearCLIP last block (attn_variant='qq', no residual/FFN) on 8 Trainium2 cores.

Data-parallel over batch N=32 -> 4 images per core. fp32r matmuls throughout.

Per image (L=577, C=1024, H=16 heads, D=64):
  z   = LayerNorm(x) (g/beta folded into W on host)
  q   = z @ Wq.T + bq ; v = z @ Wv.T + bv   (k = q, Wk unused)
  S_h = (q_h q_h^T) / sqrt(D)   (symmetric; sqrt-scale folded into Wq/bq)
  E   = exp(S);  Z_l = sum_s E[l,s]  (accumulated during exp on ACT)
  o_h^T = v_h^T E diag(1/Z)     (E symmetric -> no transpose needed)
  y   = o @ Wo.T + bo
"""
import sys

try:
    import concourse.bass as bass  # noqa: F401
except Exception:
    sys.path.insert(0, "/opt/trn_rl_repo")

import numpy as np
from contextlib import ExitStack

import concourse.bass as bass
import concourse.tile as tile
from concourse import bacc, mybir
from concourse.bass_utils import run_bass_kernel_spmd
from concourse.masks import make_identity

F32 = mybir.dt.float32
F32R = mybir.dt.float32r
AF = mybir.ActivationFunctionType
ALU = mybir.AluOpType

N_FULL, L, C = 32, 577, 1024
H, D = 16, 64
NCORES = 8
B = N_FULL // NCORES
LT = 5                 # l-tiles of 128
LP = LT * 128          # 640 (padded l for the Z row scratch only)
NPAIR = 8              # pairs of heads -> 128 partitions
SC = 320               # first N-chunk; second is 257 (both >=256 for fp32r rate)
SC2 = 258              # second matmul chunk width (even; covers l 320..578, col 577 is pad)
LW = SC + SC2          # 578 padded free width for l/s dims
LROWS = [128, 128, 128, 128, L - 512]   # valid rows per l-tile
LN_EPS = 1e-5
SCALE = float(1.0 / np.sqrt(np.sqrt(float(D))))  # folded into Wq/bq twice -> 1/sqrt(D)

TRACE = False
_CACHED = {}


def _build():
    nc = bacc.Bacc("TRN2", target_bir_lowering=False, debug=False)

    x_d = nc.dram_tensor("x_sh", [B, L, C], F32, kind="ExternalInput")
    wqT_d = nc.dram_tensor("wqT", [C, C], F32R, kind="ExternalInput")   # (c, j)
    wvT_d = nc.dram_tensor("wvT", [C, C], F32R, kind="ExternalInput")   # (c, j)
    woT_d = nc.dram_tensor("woT", [C, C], F32R, kind="ExternalInput")   # (j, c)
    bq_d = nc.dram_tensor("bq", [C], F32, kind="ExternalInput")
    bv_d = nc.dram_tensor("bv", [C], F32, kind="ExternalInput")
    bo_d = nc.dram_tensor("bo", [C], F32, kind="ExternalInput")
    y_d = nc.dram_tensor("y_sh", [B, L, C], F32, kind="ExternalOutput")

    with tile.TileContext(nc) as tc, ExitStack() as ctx:
        singles = ctx.enter_context(tc.tile_pool(name="singles", bufs=1))
        zst = ctx.enter_context(tc.tile_pool(name="zst", bufs=2))
        stats = ctx.enter_context(tc.tile_pool(name="stats", bufs=4))
        zT_p = ctx.enter_context(tc.tile_pool(name="zT", bufs=1))
        qT_p = ctx.enter_context(tc.tile_pool(name="qT", bufs=1))
        vst_p = ctx.enter_context(tc.tile_pool(name="vst", bufs=2))
        vw_p = ctx.enter_context(tc.tile_pool(name="vw", bufs=10))
        e_p = ctx.enter_context(tc.tile_pool(name="E", bufs=2))
        z_p = ctx.enter_context(tc.tile_pool(name="Z", bufs=4))
        zb_p = ctx.enter_context(tc.tile_pool(name="Zb", bufs=2))
        oT_p = ctx.enter_context(tc.tile_pool(name="oT", bufs=2))
        ow_p = ctx.enter_context(tc.tile_pool(name="oW", bufs=4))
        y_p = ctx.enter_context(tc.tile_pool(name="y", bufs=2))
        dram = ctx.enter_context(tc.tile_pool(name="dram", bufs=3, space="DRAM"))
        ps_big = ctx.enter_context(tc.tile_pool(name="ps_big", bufs=3, space="PSUM"))
        ps_s = ctx.enter_context(tc.tile_pool(name="ps_s", bufs=2, space="PSUM"))
        ps_u = ctx.enter_context(tc.tile_pool(name="ps_u", bufs=2, space="PSUM"))

        ident = singles.tile([128, 128], F32)
        make_identity(nc, ident[:])
        eps_t = singles.tile([128, 1], F32)
        nc.vector.memset(eps_t[:], LN_EPS)
        zeros_sb = singles.tile([128, LW], F32)
        nc.vector.memset(zeros_sb[:], 0.0)
        bq_sb = singles.tile([128, 8], F32)
        nc.sync.dma_start(bq_sb[:], bq_d.ap().rearrange("(t p) -> p t", p=128))
        bv_sb = singles.tile([128, C], F32)
        nc.sync.dma_start(bv_sb[:], bv_d.ap().partition_broadcast(128))
        bo_sb = singles.tile([128, C], F32)
        nc.sync.dma_start(bo_sb[:], bo_d.ap().partition_broadcast(128))
        wq_sb = singles.tile([128, 8, C], F32R)
        nc.sync.dma_start(wq_sb[:], wqT_d.ap().rearrange("(t p) j -> p t j", p=128))
        wv_sb = singles.tile([128, 8, C], F32R)
        nc.sync.dma_start(wv_sb[:], wvT_d.ap().rearrange("(t p) j -> p t j", p=128))
        wo_sb = singles.tile([128, 8, C], F32R)
        nc.sync.dma_start(wo_sb[:], woT_d.ap().rearrange("(t p) c -> p t c", p=128))

        for b in range(B):
            # ---------------- LayerNorm ----------------
            z_tiles = []
            for lt in range(LT):
                rows = LROWS[lt]
                xz = zst.tile([128, C], F32)
                if rows < 128:
                    nc.vector.memset(xz[:], 0.0)
                nc.sync.dma_start(xz[:rows, :], x_d.ap()[b, 128 * lt:128 * lt + rows, :])
                st = stats.tile([128, 2, 6], F32)
                nc.vector.bn_stats(st[:, 0, :], xz[:, 0:512])
                nc.vector.bn_stats(st[:, 1, :], xz[:, 512:1024])
                mv = stats.tile([128, 2], F32)
                nc.vector.bn_aggr(mv[:], st[:])
                rstd = stats.tile([128, 1], F32)
                nc.scalar.activation(out=rstd[:], in_=mv[:, 1:2], func=AF.Sqrt,
                                     bias=eps_t[:], scale=1.0)
                nc.vector.reciprocal(rstd[:], rstd[:])
                nc.vector.tensor_scalar(out=xz[:], in0=xz[:], scalar1=mv[:, 0:1],
                                        scalar2=rstd[:], op0=ALU.subtract, op1=ALU.mult)
                z_tiles.append(xz)

            # ---------------- transpose z -> zT [c, l] ----------------
            zT = zT_p.tile([128, 8, LW], F32R)
            nc.vector.tensor_copy(zT[:, :, L:LW], zeros_sb[:, 0:8].unsqueeze(2))
            for lt in range(LT):
                rows = LROWS[lt]
                for ct in range(8):
                    pt = ps_big.tile([128, 512], F32, tag="mm")
                    nc.tensor.transpose(pt[:, 0:128], z_tiles[lt][:, 128 * ct:128 * (ct + 1)], ident[:])
                    nc.vector.tensor_copy(zT[:, ct, 128 * lt:128 * lt + rows], pt[:, 0:rows])

            # ---------------- qT = Wq' z^T + bq' [j, l] ----------------
            qT = qT_p.tile([128, 8, LW], F32R)
            for jt in range(8):
                for k, (c0, cw) in enumerate(((0, SC), (SC, SC2))):
                    pq = ps_big.tile([128, 512], F32, tag="mm")
                    for ct in range(8):
                        nc.tensor.matmul(pq[:, 0:cw], wq_sb[:, ct, 128 * jt:128 * (jt + 1)],
                                         zT[:, ct, c0:c0 + cw],
                                         start=(ct == 0), stop=(ct == 7))
                    nc.scalar.activation(out=qT[:, jt, c0:c0 + cw], in_=pq[:, 0:cw],
                                         func=AF.Identity, bias=bq_sb[:, jt:jt + 1], scale=1.0)

            # ---------------- v = z Wv'^T + bv' [s, j] -> DRAM ----------------
            v_dram = dram.tile([LT, 128, C], F32R)
            for lt in range(LT):
                rows = LROWS[lt]
                for jc in range(2):
                    pv = ps_big.tile([128, 512], F32, tag="mm")
                    for ct in range(8):
                        nc.tensor.matmul(pv[0:rows, :], zT[:, ct, 128 * lt:128 * lt + rows],
                                         wv_sb[:, ct, 512 * jc:512 * (jc + 1)],
                                         start=(ct == 0), stop=(ct == 7))
                    stg = vst_p.tile([128, 512], F32R)
                    if rows < 128:
                        nc.vector.tensor_copy(stg[64:128, :], zeros_sb[64:128, 0:512])
                    nc.vector.tensor_add(stg[0:rows, :], pv[0:rows, :],
                                         bv_sb[0:rows, 512 * jc:512 * (jc + 1)])
                    nc.sync.dma_start(v_dram[lt, :, 512 * jc:512 * (jc + 1)], stg[:])

            # ---------------- attention, one head-pair at a time ----------------
            oT_dram = dram.tile([NPAIR, 128, L], F32R)
            for t in range(NPAIR):
                E_AB = [e_p.tile([128, LT, LW], F32R, tag="E_AB", name=f"E{t}_{hh}")
                        for hh in range(2)]
                Zp = z_p.tile([128, 2, 2, LT], F32)
                nc.vector.memset(Zp[:], 0.0)
                for h in range(2):
                    nc.vector.tensor_copy(E_AB[h][64:128, 4, :], zeros_sb[64:128, 0:LW])
                    nc.vector.tensor_copy(E_AB[h][:, :, L:LW], zeros_sb[:, 0:LT].unsqueeze(2))
                    p0 = 64 * h
                    for lt in range(LT):
                        rows = LROWS[lt]
                        for k, (c0, cw) in enumerate(((0, SC), (SC, SC2))):
                            cwe = min(cw, L - c0)   # exp/accum only over valid cols
                            pS = ps_s.tile([128, 512], F32)
                            nc.tensor.matmul(pS[0:rows, 0:cw],
                                             qT[p0:p0 + 64, t, 128 * lt:128 * lt + rows],
                                             qT[p0:p0 + 64, t, c0:c0 + cw],
                                             start=True, stop=True)
                            nc.scalar.activation(out=E_AB[h][0:rows, lt, c0:c0 + cwe],
                                                 in_=pS[0:rows, 0:cwe], func=AF.Exp,
                                                 accum_out=Zp[0:rows, h, k, lt:lt + 1])
                Z2 = z_p.tile([128, 10], F32)
                nc.vector.tensor_tensor(Z2[:, 0:5], Zp[:, 0, 0, :], Zp[:, 0, 1, :], ALU.add)
                nc.vector.tensor_tensor(Z2[:, 5:10], Zp[:, 1, 0, :], Zp[:, 1, 1, :], ALU.add)
                nc.vector.reciprocal(Z2[:], Z2[:])
                zrow_d = dram.tile([2, LP], F32)
                nc.sync.dma_start(zrow_d[0, :].rearrange("(t p) -> p t", p=128), Z2[:, 0:5])
                nc.sync.dma_start(zrow_d[1, :].rearrange("(t p) -> p t", p=128), Z2[:, 5:10])
                zb = zb_p.tile([64, 2, LW], F32)
                nc.sync.dma_start(zb[:, 0, :], zrow_d[0, 0:LW].partition_broadcast(64))
                nc.sync.dma_start(zb[:, 1, :], zrow_d[1, 0:LW].partition_broadcast(64))

                vw = []
                for st in range(LT):
                    w = vw_p.tile([128, 128], F32R)
                    nc.sync.dma_start(w[:], v_dram[st, :, 128 * t:128 * (t + 1)])
                    vw.append(w)

                for h in range(2):
                    oTh = oT_p.tile([64, LW], F32R, tag="oT", name=f"oT{t}_{h}")
                    for k, (c0, cw) in enumerate(((0, SC), (SC, SC2))):
                        pU = ps_u.tile([64, 512], F32, tag="pU")
                        for st in range(LT):
                            nc.tensor.matmul(pU[:, 0:cw],
                                             vw[st][:, 64 * h:64 * h + 64],
                                             E_AB[h][:, st, c0:c0 + cw],
                                             start=(st == 0), stop=(st == LT - 1))
                        nc.vector.tensor_mul(oTh[:, c0:c0 + cw], pU[:, 0:cw], zb[:, h, c0:c0 + cw])
                    nc.sync.dma_start(oT_dram[t, 64 * h:64 * h + 64, :], oTh[:, 0:L])

            # ---------------- y = o Wo^T + bo ----------------
            for lt in range(LT):
                rows = LROWS[lt]
                ow = []
                for jt in range(8):
                    w = ow_p.tile([128, 128], F32R)
                    nc.sync.dma_start(w[:, 0:rows], oT_dram[jt, :, 128 * lt:128 * lt + rows])
                    ow.append(w)
                y_sb = y_p.tile([128, C], F32)
                for cc in range(2):
                    py = ps_big.tile([128, 512], F32, tag="mm")
                    for jt in range(8):
                        nc.tensor.matmul(py[0:rows, :], ow[jt][:, 0:rows],
                                         wo_sb[:, jt, 512 * cc:512 * (cc + 1)],
                                         start=(jt == 0), stop=(jt == 7))
                    nc.vector.tensor_add(y_sb[0:rows, 512 * cc:512 * (cc + 1)], py[0:rows, :],
                                         bo_sb[0:rows, 512 * cc:512 * (cc + 1)])
                nc.sync.dma_start(y_d.ap()[b, 128 * lt:128 * lt + rows, :], y_sb[0:rows, :])

    nc.compile()
    return nc


def _prep_inputs(ln1_g, ln1_b, w_in, b_in, w_out, b_out):
    g = np.asarray(ln1_g, np.float32)
    beta = np.asarray(ln1_b, np.float32)
    w_in = np.asarray(w_in, np.float32)
    b_in = np.asarray(b_in, np.float32)
    wq = w_in[0:C] * g[None, :]
    wv = w_in[2 * C:3 * C] * g[None, :]
    bq = (wq @ beta + b_in[0:C]).astype(np.float32)
    bv = (wv @ beta + b_in[2 * C:3 * C]).astype(np.float32)
    return {
        "wqT": np.ascontiguousarray((wq.T * SCALE).astype(np.float32)),
        "wvT": np.ascontiguousarray(wv.T).astype(np.float32),
        "woT": np.ascontiguousarray(np.asarray(w_out, np.float32).T),
        "bq": (bq * SCALE).astype(np.float32),
        "bv": bv.astype(np.float32),
        "bo": np.asarray(b_out, np.float32),
    }


def kernel(x, ln1_g, ln1_b, w_in, b_in, w_out, b_out):
    x = np.asarray(x, np.float32)
    shared = _prep_inputs(ln1_g, ln1_b, w_in, b_in, w_out, b_out)

    if "nc" not in _CACHED:
        _CACHED["nc"] = _build()
    nc = _CACHED["nc"]

    in_maps = []
    for i in range(NCORES):
        m = dict(shared)
        m["x_sh"] = np.ascontiguousarray(x[B * i:B * (i + 1)])
        in_maps.append(m)

    res = run_bass_kernel_spmd(nc, in_maps, core_ids=list(range(NCORES)), trace=TRACE)
    _CACHED["last_result"] = res
    y = np.concatenate([res.results[i]["y_sh"] for i in range(NCORES)], axis=0)
    return y.astype(np.float32)
